# revision 7
# baseline (speedup 1.0000x reference)
"""ConvLinear4bit (QuaRot-style W4A4 linear) on 8 Trainium2 cores.

Column-parallel: each core holds a 512-row shard of weight_int/w_scale/bias,
computes out[:, shard] for the full batch, host concatenates shards.

Per-core pipeline (all tiles of 128 rows, fully unrolled):
  1. DMA x rows (fp32)                                  [natural: n on partitions]
  2. PE-transpose x tiles (fp32r, 1.5 cyc/row)          -> xT [i on partitions]
  3. Rotation: psum += xT_chunk.T @ H_half (fp32r)      -> x_rot natural [n, l]
  4. ACT drains x_rot PSUM->SBUF; DVE abs-max reduce -> scales; reciprocal
  5. ACT: t = x_rot * (7/amax) + 2^23  (fused scale+round-to-nearest-even)
     DVE: q = (t - 2^23) cast bf16     (exact small ints)
  6. xbar DMA-transpose q (bf16)                        -> qT [j on partitions]
  7. GEMM: psum += qT_tile.T @ WT_tile (bf16, exact int arithmetic in fp32 psum)
  8. DVE epilogue: (acc * s_x) * w_scale_bcast + bias_bcast -> DMA out

W prep (once): DMA int32 shard, DVE cast to bf16, PE-transpose tiles -> WT.
"""

import numpy as np

ROT = 256
N_FULL, D_FULL, DOUT_FULL, NCORES = 4096, 4096, 4096, 8

_MAGIC = 12582912.0  # 1.5*2^23: fp32 add/sub forces round-to-nearest-even at int granularity


def _hadamard(n):
    h = np.array([[1.0]], dtype=np.float32)
    while h.shape[0] < n:
        h = np.block([[h, h], [h, -h]]).astype(np.float32)
    return (h / np.sqrt(n)).astype(np.float32)


def build_program(N, D, OSH):
    """Build the SPMD single-core program. N rows, D in-features, OSH out-features."""
    from concourse import bacc, mybir, tile, masks

    f32 = mybir.dt.float32
    f32r = mybir.dt.float32r
    bf16 = mybir.dt.bfloat16
    i32 = mybir.dt.int32

    NT = N // 128       # n row-tiles
    KT = D // 128       # contraction tiles
    KB = D // ROT       # hadamard blocks
    OT = OSH // 128     # out-feature tiles (shard)

    nc = bacc.Bacc(None, target_bir_lowering=False, debug=False)
    x_d = nc.declare_dram_parameter("x", [N, D], f32, isOutput=False)
    w_d = nc.declare_dram_parameter("w", [OSH, D], i32, isOutput=False)
    ws_d = nc.declare_dram_parameter("ws", [1, OSH], f32, isOutput=False)
    b_d = nc.declare_dram_parameter("b", [1, OSH], f32, isOutput=False)
    h_d = nc.declare_dram_parameter("h", [ROT, ROT], f32, isOutput=False)
    out_d = nc.declare_dram_parameter("out", [N, OSH], f32, isOutput=True)

    with tile.TileContext(nc) as tc:
        with (
            tc.tile_pool(name="const", bufs=1) as const,
            tc.tile_pool(name="wt", bufs=1) as wtp,
            tc.tile_pool(name="xb", bufs=2) as xbp,
            tc.tile_pool(name="xt", bufs=2) as xtp,
            tc.tile_pool(name="xr", bufs=2) as xrp,
            tc.tile_pool(name="q", bufs=2) as qp,
            tc.tile_pool(name="qt", bufs=2) as qtp,
            tc.tile_pool(name="o", bufs=2) as op,
            tc.tile_pool(name="s", bufs=4) as sp,
            tc.tile_pool(name="pT", bufs=3, space="PSUM") as pTp,
            tc.tile_pool(name="pR", bufs=3, space="PSUM") as pRp,
            tc.tile_pool(name="pA", bufs=2, space="PSUM") as pAp,
        ):
            ident = const.tile([128, 128], f32)
            masks.make_identity(nc, ident[:])
            ident_bf = const.tile([128, 128], bf16)
            masks.make_identity(nc, ident_bf[:])
            ones = const.tile([1, 128], f32)
            nc.gpsimd.memset(ones[:], 1.0)

            # H: [ROT, ROT] as [128, ROT//128, ROT]; copy to f32r via DVE
            # (walrus requires fp32r matmul operands to be rounded by a
            # compute op, not bitcast from DMA'd fp32)
            hb = const.tile([128, ROT // 128, ROT], f32)
            for h in range(ROT // 128):
                nc.sync.dma_start(hb[:, h, :], h_d[h * 128:(h + 1) * 128, :])
            hbr = const.tile([128, ROT // 128, ROT], f32r)
            nc.vector.tensor_copy(hbr[:], hb[:])

            # broadcast w_scale and bias to [128, OSH] via K=1 matmul
            wsrow = const.tile([1, 2, OSH], f32)
            nc.sync.dma_start(wsrow[:, 0, :], ws_d[:, :])
            nc.sync.dma_start(wsrow[:, 1, :], b_d[:, :])
            ws_bc = const.tile([128, OSH], f32)
            b_bc = const.tile([128, OSH], f32)
            for src, dst in ((0, ws_bc), (1, b_bc)):
                pb = pAp.tile([128, OSH], f32, tag="pA")
                nc.tensor.matmul(pb[:], ones[:], wsrow[:, src, :],
                                 start=True, stop=True)
                nc.vector.tensor_copy(dst[:], pb[:])

            # W shard -> WT bf16 [j, o] tiles (streamed in 512-feature chunks)
            WCH = 512
            wt = wtp.tile([128, KT, OSH], bf16)
            with tc.tile_pool(name="wstage", bufs=2) as wst:
                for ot in range(OT):
                    for ch in range(D // WCH):
                        wi = wst.tile([128, WCH], i32, tag="wi")
                        nc.sync.dma_start(
                            wi[:], w_d[ot * 128:(ot + 1) * 128,
                                       ch * WCH:(ch + 1) * WCH])
                        wb = wst.tile([128, WCH], bf16, tag="wb")
                        nc.vector.tensor_copy(wb[:], wi[:])
                        for k in range(WCH // 128):
                            kt = ch * (WCH // 128) + k
                            pw = pTp.tile([128, 128], bf16, tag="pT")
                            nc.tensor.matmul(pw[:], wb[:, k * 128:(k + 1) * 128],
                                             ident_bf[:], is_transpose=True)
                            nc.scalar.copy(wt[:, kt, ot * 128:(ot + 1) * 128], pw[:])

            # main loop over row tiles
            for t in range(NT):
                xb = xbp.tile([128, D], f32, tag="xb")
                nc.sync.dma_start(xb[:], x_d[t * 128:(t + 1) * 128, :])

                # transpose x tile (fp32); drain copy rounds to f32r
                xT = xtp.tile([128, KT, 128], f32r, tag="xt")
                for kt in range(KT):
                    pT = pTp.tile([128, 128], f32, tag="pT")
                    nc.tensor.matmul(pT[:], xb[:, kt * 128:(kt + 1) * 128],
                                     ident[:], is_transpose=True)
                    nc.vector.tensor_copy(xT[:, kt, :], pT[:])

                # rotation: x_rot[n, l] per 256-block
                xr = xrp.tile([128, D], f32, tag="xr")
                for b in range(KB):
                    pr = pRp.tile([128, ROT], f32, tag="pR")
                    for h in range(ROT // 128):
                        nc.tensor.matmul(pr[:], xT[:, 2 * b + h, :],
                                         hbr[:, h, :],
                                         start=(h == 0), stop=(h == ROT // 128 - 1))
                    nc.scalar.copy(xr[:, b * ROT:(b + 1) * ROT], pr[:])

                # scales: amax over row, recip7 = 7/amax, sx = amax/7
                ms = sp.tile([128, KB], f32, tag="ms")
                for b in range(KB):
                    nc.vector.tensor_reduce(
                        ms[:, b:b + 1], xr[:, b * ROT:(b + 1) * ROT],
                        axis=mybir.AxisListType.X, op=mybir.AluOpType.max,
                        apply_absolute_value=True)
                amax = sp.tile([128, 1], f32, tag="amax")
                nc.vector.tensor_reduce(amax[:], ms[:],
                                        axis=mybir.AxisListType.X,
                                        op=mybir.AluOpType.max)
                recip7 = sp.tile([128, 1], f32, tag="recip7")
                nc.vector.reciprocal(recip7[:], amax[:])
                nc.vector.tensor_scalar(recip7[:], recip7[:], 7.0, None,
                                        mybir.AluOpType.mult)
                sx = sp.tile([128, 1], f32, tag="sx")
                nc.vector.tensor_scalar(sx[:], amax[:], 1.0 / 7.0, None,
                                        mybir.AluOpType.mult)

                # quantize: in-place t = xr*recip7 + 2^23 (ACT), q = t - 2^23 (DVE, bf16)
                nc.scalar.activation(xr[:], xr[:],
                                     mybir.ActivationFunctionType.Copy,
                                     bias=_MAGIC, scale=recip7[:])
                q = qp.tile([128, D], bf16, tag="q")
                nc.vector.tensor_scalar(q[:], xr[:], _MAGIC, None,
                                        mybir.AluOpType.subtract)

                # qT via xbar transpose (bf16): [128, D] -> [128, KT, 128]
                qT = qtp.tile([128, KT, 128], bf16, tag="qt")
                nc.sync.dma_start_transpose(qT[:], q[:])

                # GEMM: acc[n, o] += qT_kt.T @ WT_kt
                pa = pAp.tile([128, OSH], f32, tag="pA")
                for kt in range(KT):
                    nc.tensor.matmul(pa[:], qT[:, kt, :], wt[:, kt, :],
                                     start=(kt == 0), stop=(kt == KT - 1))

                # epilogue: (acc * sx) * ws + b
                ot_t = op.tile([128, OSH], f32, tag="o")
                nc.vector.scalar_tensor_tensor(ot_t[:], pa[:], sx[:], ws_bc[:],
                                               op0=mybir.AluOpType.mult,
                                               op1=mybir.AluOpType.mult)
                nc.vector.tensor_tensor(ot_t[:], ot_t[:], b_bc[:],
                                        op=mybir.AluOpType.add)
                nc.sync.dma_start(out_d[t * 128:(t + 1) * 128, :], ot_t[:])

    nc.compile()
    return nc


_CACHE = {}


def _get_program(N, D, OSH):
    key = (N, D, OSH)
    if key not in _CACHE:
        _CACHE[key] = build_program(N, D, OSH)
    return _CACHE[key]


def kernel(x, weight_int, w_scale, bias):
    from concourse.bass_utils import run_bass_kernel_spmd

    x = np.ascontiguousarray(np.asarray(x, dtype=np.float32))
    weight_int = np.ascontiguousarray(np.asarray(weight_int, dtype=np.int32))
    w_scale = np.asarray(w_scale, dtype=np.float32).reshape(-1)
    bias = np.asarray(bias, dtype=np.float32).reshape(-1)

    N, D = x.shape
    DOUT = weight_int.shape[0]
    OSH = DOUT // NCORES
    H = _hadamard(ROT)

    nc = _get_program(N, D, OSH)
    in_maps = []
    for c in range(NCORES):
        sl = slice(c * OSH, (c + 1) * OSH)
        in_maps.append({
            "x": x,
            "w": weight_int[sl, :],
            "ws": w_scale[sl].reshape(1, OSH),
            "b": bias[sl].reshape(1, OSH),
            "h": H,
        })
    res = run_bass_kernel_spmd(nc, in_maps, list(range(NCORES)))
    out = np.concatenate([res.results[c]["out"] for c in range(NCORES)], axis=1)
    return out.astype(np.float32)


# revision 12
# speedup vs baseline: 1.1201x; 1.1201x over previous
"""ConvLinear4bit (QuaRot-style W4A4 linear) on 8 Trainium2 cores.

Column-parallel: each core holds a 512-row shard of weight_int/w_scale/bias,
computes out[:, shard] for the full batch, host concatenates shards.

Per-core pipeline (all tiles of 128 rows, fully unrolled):
  1. DMA x rows (fp32)                                  [natural: n on partitions]
  2. PE-transpose x tiles (fp32r, 1.5 cyc/row)          -> xT [i on partitions]
  3. Rotation: psum += xT_chunk.T @ H_half (fp32r)      -> x_rot natural [n, l]
  4. ACT drains x_rot PSUM->SBUF; DVE abs-max reduce -> scales; reciprocal
  5. ACT: t = x_rot * (7/amax) + 2^23  (fused scale+round-to-nearest-even)
     DVE: q = (t - 2^23) cast bf16     (exact small ints)
  6. xbar DMA-transpose q (bf16)                        -> qT [j on partitions]
  7. GEMM: psum += qT_tile.T @ WT_tile (bf16, exact int arithmetic in fp32 psum)
  8. DVE epilogue: (acc * s_x) * w_scale_bcast + bias_bcast -> DMA out

W prep (once): DMA int32 shard, DVE cast to bf16, PE-transpose tiles -> WT.
"""

import numpy as np

ROT = 256
N_FULL, D_FULL, DOUT_FULL, NCORES = 4096, 4096, 4096, 8

_MAGIC = 12582912.0  # 1.5*2^23: fp32 add/sub forces round-to-nearest-even at int granularity


def _hadamard(n):
    h = np.array([[1.0]], dtype=np.float32)
    while h.shape[0] < n:
        h = np.block([[h, h], [h, -h]]).astype(np.float32)
    return (h / np.sqrt(n)).astype(np.float32)


def build_program(N, D, OSH):
    """Build the SPMD single-core program. N rows, D in-features, OSH out-features."""
    from concourse import bacc, mybir, tile, masks

    f32 = mybir.dt.float32
    f32r = mybir.dt.float32r
    bf16 = mybir.dt.bfloat16
    i32 = mybir.dt.int32

    NT = N // 128       # n row-tiles
    KT = D // 128       # contraction tiles
    KB = D // ROT       # hadamard blocks
    OT = OSH // 128     # out-feature tiles (shard)

    nc = bacc.Bacc(None, target_bir_lowering=False, debug=False)
    x_d = nc.declare_dram_parameter("x", [N, D], f32, isOutput=False)
    w_d = nc.declare_dram_parameter("w", [OSH, D], i32, isOutput=False)
    ws_d = nc.declare_dram_parameter("ws", [1, OSH], f32, isOutput=False)
    b_d = nc.declare_dram_parameter("b", [1, OSH], f32, isOutput=False)
    h_d = nc.declare_dram_parameter("h", [ROT, ROT], f32, isOutput=False)
    out_d = nc.declare_dram_parameter("out", [N, OSH], f32, isOutput=True)

    with tile.TileContext(nc) as tc:
        with (
            tc.tile_pool(name="const", bufs=1) as const,
            tc.tile_pool(name="wt", bufs=1) as wtp,
            tc.tile_pool(name="xb", bufs=2) as xbp,
            tc.tile_pool(name="xt", bufs=2) as xtp,
            tc.tile_pool(name="xr", bufs=2) as xrp,
            tc.tile_pool(name="q", bufs=2) as qp,
            tc.tile_pool(name="qt", bufs=2) as qtp,
            tc.tile_pool(name="o", bufs=2) as op,
            tc.tile_pool(name="s", bufs=4) as sp,
            tc.tile_pool(name="pT", bufs=2, space="PSUM") as pTp,
            tc.tile_pool(name="pR", bufs=2, space="PSUM") as pRp,
            tc.tile_pool(name="pA", bufs=2, space="PSUM") as pAp,
        ):
            ident = const.tile([128, 128], f32)
            masks.make_identity(nc, ident[:])
            ident_bf = const.tile([128, 128], bf16)
            masks.make_identity(nc, ident_bf[:])
            ones = const.tile([1, 128], f32)
            nc.gpsimd.memset(ones[:], 1.0)

            # H: [ROT, ROT] as [128, ROT//128, ROT]; copy to f32r via DVE
            # (walrus requires fp32r matmul operands to be rounded by a
            # compute op, not bitcast from DMA'd fp32)
            hb = const.tile([128, ROT // 128, ROT], f32)
            for h in range(ROT // 128):
                nc.sync.dma_start(hb[:, h, :], h_d[h * 128:(h + 1) * 128, :])
            hbr = const.tile([128, ROT // 128, ROT], f32r)
            nc.vector.tensor_copy(hbr[:], hb[:])

            # broadcast w_scale and bias to [128, OSH] via K=1 matmul
            wsrow = const.tile([1, 2, OSH], f32)
            nc.sync.dma_start(wsrow[:, 0, :], ws_d[:, :])
            nc.sync.dma_start(wsrow[:, 1, :], b_d[:, :])
            ws_bc = const.tile([128, OSH], f32)
            b_bc = const.tile([128, OSH], f32)
            for src, dst in ((0, ws_bc), (1, b_bc)):
                pb = pAp.tile([128, OSH], f32, tag="pA")
                nc.tensor.matmul(pb[:], ones[:], wsrow[:, src, :],
                                 start=True, stop=True)
                nc.vector.tensor_copy(dst[:], pb[:])

            # W shard -> WT bf16 [j, o] tiles (streamed in 512-feature chunks)
            WCH = 512
            wt = wtp.tile([128, KT, OSH], bf16)
            with tc.tile_pool(name="wstage", bufs=2) as wst:
                for ot in range(OT):
                    for ch in range(D // WCH):
                        wi = wst.tile([128, WCH], i32, tag="wi")
                        nc.sync.dma_start(
                            wi[:], w_d[ot * 128:(ot + 1) * 128,
                                       ch * WCH:(ch + 1) * WCH])
                        wb = wst.tile([128, WCH], bf16, tag="wb")
                        nc.vector.tensor_copy(wb[:], wi[:])
                        for k in range(WCH // 128):
                            kt = ch * (WCH // 128) + k
                            pw = pTp.tile([128, 128], bf16, tag="pT")
                            nc.tensor.matmul(pw[:], wb[:, k * 128:(k + 1) * 128],
                                             ident_bf[:], is_transpose=True)
                            nc.scalar.copy(wt[:, kt, ot * 128:(ot + 1) * 128], pw[:])

            # main loop over row tiles
            for t in range(NT):
                xb = xbp.tile([128, D], f32, tag="xb")
                nc.sync.dma_start(xb[:], x_d[t * 128:(t + 1) * 128, :])

                # transpose x tile (fp32), 8 per PSUM tile; one wide ACT
                # drain per batch rounds to f32r
                xT = xtp.tile([128, KT, 128], f32r, tag="xt")
                GB = min(8, KT)
                for g in range(KT // GB):
                    pT = pTp.tile([128, 128 * GB], f32, tag="pT")
                    for k in range(GB):
                        kt = g * GB + k
                        nc.tensor.matmul(pT[:, k * 128:(k + 1) * 128],
                                         xb[:, kt * 128:(kt + 1) * 128],
                                         ident[:], is_transpose=True)
                    nc.scalar.copy(xT[:, g * GB:(g + 1) * GB, :].rearrange(
                        "p a b -> p (a b)"), pT[:])

                # rotation: x_rot[n, l], two 256-blocks per PSUM tile,
                # one wide ACT drain per pair
                xr = xrp.tile([128, D], f32, tag="xr")
                for b in range(0, KB, 2):
                    pr = pRp.tile([128, 2 * ROT], f32, tag="pR")
                    for bb in range(2):
                        for h in range(ROT // 128):
                            nc.tensor.matmul(
                                pr[:, bb * ROT:(bb + 1) * ROT],
                                xT[:, 2 * (b + bb) + h, :], hbr[:, h, :],
                                start=(h == 0), stop=(h == ROT // 128 - 1))
                    nc.scalar.copy(xr[:, b * ROT:(b + 2) * ROT], pr[:])

                # scales: one batched abs-max reduce, then recip7 = 7/amax
                ms = sp.tile([128, KB], f32, tag="ms")
                nc.vector.tensor_reduce(
                    ms[:], xr[:].rearrange("p (b r) -> p b r", r=ROT),
                    axis=mybir.AxisListType.X, op=mybir.AluOpType.max,
                    apply_absolute_value=True)
                amax = sp.tile([128, 1], f32, tag="amax")
                nc.vector.tensor_reduce(amax[:], ms[:],
                                        axis=mybir.AxisListType.X,
                                        op=mybir.AluOpType.max)
                recip7 = sp.tile([128, 1], f32, tag="recip7")
                nc.vector.reciprocal(recip7[:], amax[:])
                nc.vector.tensor_scalar(recip7[:], recip7[:], 7.0, None,
                                        mybir.AluOpType.mult)
                sx = sp.tile([128, 1], f32, tag="sx")
                nc.vector.tensor_scalar(sx[:], amax[:], 1.0 / 7.0, None,
                                        mybir.AluOpType.mult)

                # quantize: in-place t = xr*recip7 + 2^23 (ACT), q = t - 2^23 (DVE, bf16)
                nc.scalar.activation(xr[:], xr[:],
                                     mybir.ActivationFunctionType.Copy,
                                     bias=_MAGIC, scale=recip7[:])
                q = qp.tile([128, D], bf16, tag="q")
                nc.vector.tensor_scalar(q[:], xr[:], _MAGIC, None,
                                        mybir.AluOpType.subtract)

                # qT via xbar transpose (bf16), split in halves so the GEMM
                # can start on the first half earlier
                qT = qtp.tile([128, KT, 128], bf16, tag="qt")
                half = D // 2
                for hh in range(2):
                    nc.sync.dma_start_transpose(
                        qT[:, hh * (KT // 2):(hh + 1) * (KT // 2), :],
                        q[:, hh * half:(hh + 1) * half])

                # GEMM: acc[n, o] += qT_kt.T @ WT_kt
                pa = pAp.tile([128, OSH], f32, tag="pA")
                for kt in range(KT):
                    nc.tensor.matmul(pa[:], qT[:, kt, :], wt[:, kt, :],
                                     start=(kt == 0), stop=(kt == KT - 1))

                # epilogue: (acc * sx) * ws + b
                ot_t = op.tile([128, OSH], f32, tag="o")
                nc.vector.scalar_tensor_tensor(ot_t[:], pa[:], sx[:], ws_bc[:],
                                               op0=mybir.AluOpType.mult,
                                               op1=mybir.AluOpType.mult)
                nc.vector.tensor_tensor(ot_t[:], ot_t[:], b_bc[:],
                                        op=mybir.AluOpType.add)
                nc.sync.dma_start(out_d[t * 128:(t + 1) * 128, :], ot_t[:])

    nc.compile()
    return nc


_CACHE = {}


def _get_program(N, D, OSH):
    key = (N, D, OSH)
    if key not in _CACHE:
        _CACHE[key] = build_program(N, D, OSH)
    return _CACHE[key]


def kernel(x, weight_int, w_scale, bias):
    from concourse.bass_utils import run_bass_kernel_spmd

    x = np.ascontiguousarray(np.asarray(x, dtype=np.float32))
    weight_int = np.ascontiguousarray(np.asarray(weight_int, dtype=np.int32))
    w_scale = np.asarray(w_scale, dtype=np.float32).reshape(-1)
    bias = np.asarray(bias, dtype=np.float32).reshape(-1)

    N, D = x.shape
    DOUT = weight_int.shape[0]
    OSH = DOUT // NCORES
    H = _hadamard(ROT)

    nc = _get_program(N, D, OSH)
    in_maps = []
    for c in range(NCORES):
        sl = slice(c * OSH, (c + 1) * OSH)
        in_maps.append({
            "x": x,
            "w": weight_int[sl, :],
            "ws": w_scale[sl].reshape(1, OSH),
            "b": bias[sl].reshape(1, OSH),
            "h": H,
        })
    res = run_bass_kernel_spmd(nc, in_maps, list(range(NCORES)))
    out = np.concatenate([res.results[c]["out"] for c in range(NCORES)], axis=1)
    return out.astype(np.float32)
